# revision 8
# baseline (speedup 1.0000x reference)
"""AttentionWithFastKAN Trainium2 kernel (v2).

Strategy (8 NeuronCores, data-parallel over batch):
  - Each core processes one batch element (1024 tokens) end to end.
  - All big matmuls in bf16 (weights packed bf16 host-side; halves HBM
    traffic and enables fast weight load).
  - The 54 contraction tiles (6 silu/base + 48 RBF basis) are materialized
    ONCE in SBUF as bf16 and reused by the v pass and all qk passes.
  - LayerNorm stats via ones-matmuls on PE; the stat->rsqrt chain is done
    WIDE ([128,T] after a gpsimd broadcast) instead of on one partition.
  - v pass first (token-major V), then SIX paired qk passes, one per head
    pair hp producing q(hp), k(hp) channel-major; attention for hp-1
    (AV) and hp (QK matmuls + EXP) are interleaved between passes so the
    ACT exp time hides under PE matmul time.
  - ET stored [128 keys, 512 q, 8 kt] so the softmax denominator is ONE
    DVE free-axis reduce + gpsimd partition_all_reduce (output is
    partition-replicated -> normalization needs no partition shifts);
    no colsum matmuls on PE.
  - LN2 stat matmuls are scheduled into the bubble where PE waits for the
    last head pair's EXP.
  - No max-subtraction in softmax (|S|/8 stays well inside fp32 exp range).
"""

import math

import numpy as np
import ml_dtypes

import concourse.bass as bass
import concourse.mybir as mybir
import concourse.tile as tile
from concourse import bacc
from concourse import bass_isa
from concourse.bass_utils import run_bass_kernel_spmd

F32 = mybir.dt.float32
F32R = mybir.dt.float32r
BF16 = mybir.dt.bfloat16
AF = mybir.ActivationFunctionType
AX = mybir.AxisListType
ALU = mybir.AluOpType

B, N_TOK, C = 8, 1024, 768
G = 8
H = 12
CT = C // 128               # 6 channel ptiles
NB = CT * G                 # 48 basis tiles
KT = CT + NB                # 54 contraction tiles (6 silu first, 48 basis)
GRID = np.linspace(-2.0, 2.0, G).astype(np.float64)
DENOM = 4.0 / 7.0
SQPI2 = math.sqrt(math.pi) / 2.0


def build_kernel(T=1024, sim_safe=False):
    TT = T // 128                       # token ptiles
    CHW = min(512, T)                   # matmul moving chunk width
    CH = T // CHW                       # chunks (2 at T=1024)
    VG = min(4, TT)                     # token tiles per v/proj pass
    v_passes = [list(range(i, min(i + VG, TT))) for i in range(0, TT, VG)]

    af_silu = AF.Sigmoid if sim_safe else AF.Silu
    af_derf = AF.Sigmoid if sim_safe else AF.Derivative_Erf
    af_exp = AF.Sigmoid if sim_safe else AF.Exp

    nc = bacc.Bacc("TRN2", target_bir_lowering=False, debug=False, num_devices=8)

    # ---- dram io ----
    xT_d = nc.dram_tensor("xT", (C, T), F32, kind="ExternalInput")
    w1qk_d = nc.dram_tensor("w1qk", (KT, 128, 1536), BF16, kind="ExternalInput")
    w1v_d = nc.dram_tensor("w1v", (KT, 128, 768), BF16, kind="ExternalInput")
    w2_d = nc.dram_tensor("w2", (KT, 128, 768), BF16, kind="ExternalInput")
    b1qk_d = nc.dram_tensor("b1qk", (12, 128), F32, kind="ExternalInput")
    b1v_d = nc.dram_tensor("b1v", (1, 768), F32, kind="ExternalInput")
    b2_d = nc.dram_tensor("b2", (1, 768), F32, kind="ExternalInput")
    asc1_d = nc.dram_tensor("asc1", (CT, 128), F32, kind="ExternalInput")
    abi1_d = nc.dram_tensor("abi1", (NB, 128), F32, kind="ExternalInput")
    asc2_d = nc.dram_tensor("asc2", (CT, 128), F32, kind="ExternalInput")
    abi2_d = nc.dram_tensor("abi2", (NB, 128), F32, kind="ExternalInput")
    out_d = nc.dram_tensor("out", (T, C), F32, kind="ExternalOutput")

    with tile.TileContext(nc) as tc:
        with tc.tile_pool(name="const", bufs=1) as const, \
             tc.tile_pool(name="potp", bufs=1) as potp, \
             tc.tile_pool(name="st2sb", bufs=1) as st2sb:

            # ---- constants ----
            asc1 = const.tile([128, CT], F32)
            abi1 = const.tile([128, NB], F32)
            asc2 = const.tile([128, CT], F32)
            abi2 = const.tile([128, NB], F32)
            nc.sync.dma_start(asc1[:], asc1_d.rearrange("c p -> p c"))
            nc.sync.dma_start(abi1[:], abi1_d.rearrange("k p -> p k"))
            nc.sync.dma_start(asc2[:], asc2_d.rearrange("c p -> p c"))
            nc.sync.dma_start(abi2[:], abi2_d.rearrange("k p -> p k"))
            b1qk = const.tile([128, 12], F32)
            nc.sync.dma_start(b1qk[:], b1qk_d.rearrange("o p -> p o"))
            b1v_b = const.tile([128, 768], F32)
            b2_b = const.tile([128, 768], F32)
            with tc.tile_pool(name="rowtmp", bufs=1) as rowtmp:
                b1v_row = rowtmp.tile([1, 768], F32)
                b2_row = rowtmp.tile([1, 768], F32)
                nc.sync.dma_start(b1v_row[:], b1v_d[:])
                nc.sync.dma_start(b2_row[:], b2_d[:])
                nc.gpsimd.partition_broadcast(b1v_b[:], b1v_row[:])
                nc.gpsimd.partition_broadcast(b2_b[:], b2_row[:])
            ones_bf = const.tile([128, 1], BF16)
            nc.vector.memset(ones_bf[:], 1.0)
            eps_t = const.tile([128, 1], F32)
            nc.vector.memset(eps_t[:], 1e-5)

            # ---- persistent activations ----
            OT = potp.tile([128, CT, T], BF16)        # attn out channel-major
            V = potp.tile([128, TT, 768], BF16)       # v token-major
            s2_row = st2sb.tile([1, T], F32)          # LN2 stat stash
            ss2_row = st2sb.tile([1, T], F32)

            def ln_wide_chain(s_row, ss_row, lnw):
                """[1,T] SBUF sums -> (rs_b, murs_b) [128,T] wide."""
                t1 = lnw.tile([128, T], F32, tag="t1")
                t2 = lnw.tile([128, T], F32, tag="t2")
                t3 = lnw.tile([128, T], F32, tag="t3")
                nc.gpsimd.partition_broadcast(t1[:], s_row[:])
                nc.gpsimd.partition_broadcast(t2[:], ss_row[:])
                nc.vector.tensor_scalar_mul(t1[:], t1[:], 1.0 / C)  # mean
                nc.vector.tensor_scalar_mul(t2[:], t2[:], 1.0 / C)  # E[x^2]
                nc.vector.tensor_mul(t3[:], t1[:], t1[:])
                nc.vector.tensor_sub(t2[:], t2[:], t3[:])           # var
                nc.scalar.activation(out=t3[:], in_=t2[:], func=AF.Sqrt,
                                     bias=eps_t[:, 0:1], scale=1.0)
                nc.vector.reciprocal(t2[:], t3[:])                  # rs
                nc.vector.tensor_mul(t1[:], t1[:], t2[:])           # mu*rs
                return t2, t1

            def emit_stats(src_ct, xsq, ps_s, ps_ss, first, last):
                """accumulate ones-matmul sums of src_ct and xsq [128,T]."""
                for ch in range(CH):
                    sl = slice(ch * CHW, (ch + 1) * CHW)
                    nc.tensor.matmul(ps_s[:, sl], ones_bf[:], src_ct[:, sl],
                                     start=first, stop=last)
                    nc.tensor.matmul(ps_ss[:, sl], ones_bf[:], xsq[:, sl],
                                     start=first, stop=last)

            def token_major_pass(bas, w_d, sink, wpool, ps_pool, wtag):
                """basis-stationary matmul pass: sink(tt, psum_tile)."""
                for tts in v_passes:
                    psum = {tt: ps_pool.tile([128, 768], F32, tag="psv",
                                             name=f"ps_{wtag}_{tt}")
                            for tt in tts}
                    for k in range(KT):
                        wt = wpool.tile([128, 768], BF16, tag=wtag)
                        nc.sync.dma_start(wt[:], w_d[k])
                        for tt in tts:
                            lhs = bas[:, k, tt * 128:(tt + 1) * 128]
                            nc.tensor.matmul(psum[tt][:, 0:512], lhs,
                                             wt[:, 0:512],
                                             start=(k == 0), stop=(k == KT - 1))
                            nc.tensor.matmul(psum[tt][:, 512:768], lhs,
                                             wt[:, 512:768],
                                             start=(k == 0), stop=(k == KT - 1))
                    for tt in tts:
                        sink(tt, psum[tt])

            def build_basis(src, bas, asc, abi, hsrc):
                """silu + basis tiles from src/hsrc [128,CT,T] into bas."""
                for kb in range(NB):
                    ct = kb % CT
                    nc.scalar.activation(out=bas[:, CT + kb], in_=hsrc[:, ct],
                                         func=af_derf,
                                         scale=asc[:, ct:ct + 1],
                                         bias=abi[:, kb:kb + 1])

            # ================= layer 1: LN + silu + basis =================
            bas1_pool = tc.tile_pool(name="bas1", bufs=1)
            bas1p = bas1_pool.__enter__()
            bas1 = bas1p.tile([128, KT, T], BF16)
            with tc.tile_pool(name="xload", bufs=1) as xpool, \
                 tc.tile_pool(name="ln1tmp", bufs=2) as ln1tmp, \
                 tc.tile_pool(name="ln1w", bufs=1) as ln1w, \
                 tc.tile_pool(name="ps_st1", bufs=1, space="PSUM") as ps_st1:
                xT = xpool.tile([128, CT, T], F32)
                xr6 = xT_d.rearrange("(ct p) t -> ct p t", p=128)
                for ct in range(CT):
                    nc.sync.dma_start(xT[:, ct], xr6[ct])
                # silu first: lets the v pass start before the LN chain ends
                for ct in range(CT):
                    nc.scalar.activation(out=bas1[:, ct], in_=xT[:, ct],
                                         func=af_silu)
                ps_s = ps_st1.tile([1, T], F32, tag="ps_s")
                ps_ss = ps_st1.tile([1, T], F32, tag="ps_ss")
                for ct in range(CT):
                    xr = ln1tmp.tile([128, T], BF16, tag="xr")
                    nc.vector.tensor_copy(xr[:], xT[:, ct])
                    xsq = ln1tmp.tile([128, T], BF16, tag="xsq")
                    nc.vector.tensor_mul(xsq[:], xT[:, ct], xT[:, ct])
                    emit_stats(xr, xsq, ps_s, ps_ss, ct == 0, ct == CT - 1)
                s_row = ln1w.tile([1, T], F32, tag="s_row")
                ss_row = ln1w.tile([1, T], F32, tag="ss_row")
                nc.vector.tensor_copy(s_row[:], ps_s[:])
                nc.vector.tensor_copy(ss_row[:], ps_ss[:])
                rs_b, murs_b = ln_wide_chain(s_row, ss_row, ln1w)
                # hT in place of xT (silu + stats already consumed x)
                for ct in range(CT):
                    nc.vector.tensor_mul(xT[:, ct], xT[:, ct], rs_b[:])
                    nc.vector.tensor_sub(xT[:, ct], xT[:, ct], murs_b[:])
                build_basis(xT, bas1, asc1, abi1, xT)

            # ================= v pass =================
            with tc.tile_pool(name="w1vs", bufs=8) as w1vs, \
                 tc.tile_pool(name="ps_v", bufs=4, space="PSUM") as ps_v:
                def v_sink(tt, ps):
                    nc.vector.tensor_add(V[:, tt], ps[:], b1v_b[:])
                token_major_pass(bas1, w1v_d, v_sink, w1vs, ps_v, "w1v")

            # ================= qk passes + attention =================
            with tc.tile_pool(name="qkTp", bufs=1) as qkTp, \
                 tc.tile_pool(name="ETp", bufs=1) as ETp, \
                 tc.tile_pool(name="denp", bufs=1) as denp, \
                 tc.tile_pool(name="w1s", bufs=8) as w1s, \
                 tc.tile_pool(name="ln2tmp", bufs=1) as ln2tmp, \
                 tc.tile_pool(name="ps_at", bufs=2, space="PSUM") as ps_at, \
                 tc.tile_pool(name="ps_av", bufs=2, space="PSUM") as ps_av:

                def emit_av(hp, ET):
                    """A@V for head pair hp + normalize into OT[:, hp]."""
                    for c in range(CH):
                        po = ps_av.tile([128, CHW], F32, tag="psav",
                                        name=f"psav_{hp}_{c}")
                        for hh in (0, 1):
                            h = 2 * hp + hh
                            bp = hh * 64
                            for kt in range(TT):
                                nc.tensor.matmul(
                                    po[bp:bp + 64],
                                    V[:, kt, h * 64:(h + 1) * 64],
                                    ET[(hh, c)][:, :, kt],
                                    start=(kt == 0), stop=(kt == TT - 1))
                        for hh in (0, 1):
                            bp = hh * 64
                            den = denp.tile([128, CHW], F32, tag=f"den{hh}")
                            nc.vector.tensor_reduce(
                                den[:], ET[(hh, c)][:], axis=AX.X,
                                op=ALU.add)
                            allr = denp.tile([128, CHW], F32, tag=f"allr{hh}")
                            nc.gpsimd.partition_all_reduce(
                                allr[:], den[:], channels=128,
                                reduce_op=bass_isa.ReduceOp.add)
                            nc.vector.reciprocal(allr[bp:bp + 64],
                                                 allr[bp:bp + 64])
                            nc.vector.tensor_mul(
                                OT[bp:bp + 64, hp, c * CHW:(c + 1) * CHW],
                                po[bp:bp + 64], allr[bp:bp + 64])

                prev = None
                with tc.tile_pool(name="ps_qk", bufs=4, space="PSUM") as ps_qk:
                    for hp in range(6):
                        qkT = qkTp.tile([128, 2, T], F32R, tag="qkT",
                                        name=f"qkT_{hp}")
                        ps = {(j, c): ps_qk.tile([128, CHW], F32, tag="psqk",
                                                 name=f"psqk_{hp}_{j}_{c}")
                              for j in (0, 1) for c in range(CH)}
                        for k in range(KT):
                            wt = w1s.tile([128, 256], BF16, tag="w1t")
                            nc.sync.dma_start(
                                wt[:], w1qk_d[k, :, hp * 256:(hp + 1) * 256])
                            for j in (0, 1):
                                for c in range(CH):
                                    nc.tensor.matmul(
                                        ps[(j, c)][:],
                                        wt[:, j * 128:(j + 1) * 128],
                                        bas1[:, k, c * CHW:(c + 1) * CHW],
                                        start=(k == 0), stop=(k == KT - 1))
                        for j in (0, 1):
                            for c in range(CH):
                                nc.vector.tensor_scalar_add(
                                    qkT[:, j, c * CHW:(c + 1) * CHW],
                                    ps[(j, c)][:],
                                    b1qk[:, 2 * hp + j:2 * hp + j + 1])
                        # AV for previous pair (its EXP ran during this pass)
                        if prev is not None:
                            emit_av(*prev)
                        # QK + EXP for this pair
                        ET = {(hh, c): ETp.tile([128, CHW, TT], BF16,
                                                tag=f"ET{hh}{c}",
                                                name=f"ET_{hp}_{hh}_{c}")
                              for hh in (0, 1) for c in range(CH)}
                        for mt in range(TT):
                            for c in range(CH):
                                pst = {hh: ps_at.tile(
                                           [128, CHW], F32, tag="psat",
                                           name=f"psat_{hp}_{mt}_{c}_{hh}")
                                       for hh in (0, 1)}
                                for hh in (0, 1):
                                    bp = hh * 64
                                    nc.tensor.matmul(
                                        pst[hh][:],
                                        qkT[bp:bp + 64, 1,
                                            mt * 128:(mt + 1) * 128],
                                        qkT[bp:bp + 64, 0,
                                            c * CHW:(c + 1) * CHW],
                                        start=True, stop=True)
                                for hh in (0, 1):
                                    nc.scalar.activation(
                                        out=ET[(hh, c)][:, :, mt],
                                        in_=pst[hh][:], func=af_exp,
                                        scale=0.125)
                        prev = (hp, ET)

                # tail: LN2 stats fill the bubble while EXP(5) finishes
                with tc.tile_pool(name="ps_st2", bufs=1,
                                  space="PSUM") as ps_st2:
                    ps2_s = ps_st2.tile([1, T], F32, tag="ps2_s")
                    ps2_ss = ps_st2.tile([1, T], F32, tag="ps2_ss")

                    def emit_stats2(hp):
                        sq = ln2tmp.tile([128, T], BF16, tag="sq")
                        nc.vector.tensor_mul(sq[:], OT[:, hp], OT[:, hp])
                        emit_stats(OT[:, hp], sq, ps2_s, ps2_ss,
                                   hp == 0, hp == 5)

                    for hp in range(5):
                        emit_stats2(hp)
                    emit_av(*prev)
                    emit_stats2(5)
                    nc.vector.tensor_copy(s2_row[:], ps2_s[:])
                    nc.vector.tensor_copy(ss2_row[:], ps2_ss[:])

            bas1_pool.__exit__(None, None, None)

            # ================= layer 2 (proj) =================
            with tc.tile_pool(name="ln2w", bufs=1) as ln2w:
                rs2_b, murs2_b = ln_wide_chain(s2_row, ss2_row, ln2w)
                with tc.tile_pool(name="bas2", bufs=1) as bas2p:
                    bas2 = bas2p.tile([128, KT, T], BF16)
                    with tc.tile_pool(name="hT2p", bufs=1) as hT2p:
                        hT2 = hT2p.tile([128, CT, T], F32)
                        for ct in range(CT):
                            nc.scalar.activation(out=bas2[:, ct],
                                                 in_=OT[:, ct],
                                                 func=af_silu)
                            nc.vector.tensor_mul(hT2[:, ct], OT[:, ct],
                                                 rs2_b[:])
                            nc.vector.tensor_sub(hT2[:, ct], hT2[:, ct],
                                                 murs2_b[:])
                        build_basis(OT, bas2, asc2, abi2, hT2)

                    with tc.tile_pool(name="w2s", bufs=8) as w2s, \
                         tc.tile_pool(name="outst", bufs=3) as outst, \
                         tc.tile_pool(name="ps_p", bufs=4,
                                      space="PSUM") as ps_p:
                        out_r = out_d.rearrange("(tt p) o -> tt p o", p=128)

                        def p_sink(tt, ps):
                            ob = outst.tile([128, 768], F32, tag="ob")
                            nc.vector.tensor_add(ob[:], ps[:], b2_b[:])
                            nc.sync.dma_start(out_r[tt], ob[:])
                        token_major_pass(bas2, w2_d, p_sink, w2s, ps_p, "w2")

    nc.compile()
    return nc


def host_prep(inputs, T=1024):
    """Build per-core input maps from the full (unsharded) inputs."""
    x = np.asarray(inputs["x"], dtype=np.float32)

    def pack_layer(spline_w, base_w, ln_w, ln_b):
        spline_w = np.asarray(spline_w, dtype=np.float64)
        base_w = np.asarray(base_w, dtype=np.float64)
        O = spline_w.shape[1]
        W = np.empty((KT, 128, O), dtype=np.float64)
        for ct in range(CT):
            W[ct] = base_w[ct * 128:(ct + 1) * 128]
        for g in range(G):
            sg = spline_w[g::G] * SQPI2          # [768, O]
            for ct in range(CT):
                W[CT + g * CT + ct] = sg[ct * 128:(ct + 1) * 128]
        ln_w = np.asarray(ln_w, dtype=np.float64)
        ln_b = np.asarray(ln_b, dtype=np.float64)
        asc = (ln_w / DENOM).reshape(CT, 128).astype(np.float32)
        abi = np.empty((NB, 128), dtype=np.float32)
        for g in range(G):
            for ct in range(CT):
                abi[g * CT + ct] = \
                    ((ln_b - GRID[g]) / DENOM)[ct * 128:(ct + 1) * 128]
        return W.astype(ml_dtypes.bfloat16), asc, abi

    W1, asc1, abi1 = pack_layer(inputs["qkv_spline_w"], inputs["qkv_base_w"],
                                inputs["qkv_ln_w"], inputs["qkv_ln_b"])
    W2, asc2, abi2 = pack_layer(inputs["proj_spline_w"], inputs["proj_base_w"],
                                inputs["proj_ln_w"], inputs["proj_ln_b"])
    b1 = np.asarray(inputs["qkv_base_b"], dtype=np.float32)
    b2 = np.asarray(inputs["proj_base_b"], dtype=np.float32)

    # paired qk weight layout: [q(hp), k(hp)] contiguous per head pair
    w1qk = np.empty((KT, 128, 1536), dtype=ml_dtypes.bfloat16)
    b1qk = np.empty((12, 128), dtype=np.float32)
    for hp in range(6):
        w1qk[:, :, hp * 256:hp * 256 + 128] = \
            W1[:, :, hp * 128:(hp + 1) * 128]
        w1qk[:, :, hp * 256 + 128:(hp + 1) * 256] = \
            W1[:, :, 768 + hp * 128:768 + (hp + 1) * 128]
        b1qk[2 * hp] = b1[hp * 128:(hp + 1) * 128]
        b1qk[2 * hp + 1] = b1[768 + hp * 128:768 + (hp + 1) * 128]

    shared = {
        "w1qk": w1qk,
        "w1v": np.ascontiguousarray(W1[:, :, 1536:]),
        "w2": np.ascontiguousarray(W2),
        "b1qk": b1qk,
        "b1v": b1[1536:].reshape(1, 768).copy(),
        "b2": b2.reshape(1, 768).copy(),
        "asc1": asc1, "abi1": abi1, "asc2": asc2, "abi2": abi2,
    }
    in_maps = []
    for core in range(x.shape[0]):
        m = dict(shared)
        m["xT"] = np.ascontiguousarray(x[core, :T].T)
        in_maps.append(m)
    return in_maps


_NC_CACHE = {}


def _get_nc(T=1024, sim_safe=False):
    key = (T, sim_safe)
    if key not in _NC_CACHE:
        _NC_CACHE[key] = build_kernel(T, sim_safe=sim_safe)
    return _NC_CACHE[key]


def kernel(**inputs) -> np.ndarray:
    nc = _get_nc()
    in_maps = host_prep(inputs)
    res = run_bass_kernel_spmd(nc, in_maps, core_ids=list(range(8)))
    out = np.stack([res.results[c]["out"] for c in range(len(in_maps))])
    return out.astype(np.float32)


if __name__ == "__main__":
    data = np.load("/root/problem/ref_data.npz")
    inputs = {k[3:]: data[k] for k in data.files if k.startswith("in_")}
    expected = data["expected64"]
    actual = kernel(**inputs)
    err = np.abs(actual - expected)
    print("absmax err:", err.max(),
          "rel2max:", err.max() / np.abs(expected).max())
    print("rel l2:",
          np.linalg.norm(actual - expected) / np.linalg.norm(expected))


# revision 22
# speedup vs baseline: 1.2767x; 1.2767x over previous
"""AttentionWithFastKAN Trainium2 kernel (v3).

Strategy (8 NeuronCores, data-parallel over batch):
  - Each core processes one batch element (1024 tokens) end to end.
  - FastKAN basis computed on ScalarE as Derivative_Erf(scale*h+bias)
    (= 2/sqrt(pi)*exp(-u^2); sqrt(pi)/2 folded into spline weights
    host-side), recomputed per matmul pass in f32r for full precision
    (bf16 basis loses ~0.1% which sharp softmax logits amplify ~|S|x).
  - All matmuls f32r (1 cycle/row at moving>=256, same speed as bf16).
  - LayerNorm stats via ones-matmuls on PE; the stat->rsqrt chain runs
    WIDE ([128,T] after gpsimd broadcast), not on one partition.
  - Order: LN1 -> v pass (token-major V) -> 3 qk passes of 4 output
    tiles (q/k for head pairs 2i, 2i+1, weights packed so the pass reads
    one contiguous 512-col slice).  Between passes an attention window
    runs: AV for the previous pair-set (interleaved with) QK matmuls for
    the current one; EXPs execute on ACT during the window and trail into
    the next pass's silu-tile matmuls (k order puts the 6 silu tiles
    first, so the pass start has no ACT dependency).
  - ET stored [128 keys, TT, 512 q] (contiguous EXP writes - strided
    writes cost ~3x on ACT).  Softmax denominator = 7 DVE adds over kt +
    gpsimd partition_all_reduce (partition-replicated output, so
    normalization needs no partition shifts); no colsum matmuls on PE.
  - LN2 stat matmuls scheduled into the bubble where PE waits for the
    last pair-set's EXP; the wide LN2 chain runs after stats are stashed
    to SBUF, then proj mirrors the v pass.
  - No max-subtraction in softmax (|S|/8 stays well inside fp32 range).
"""

import math

import numpy as np

import concourse.bass as bass
import concourse.mybir as mybir
import concourse.tile as tile
from concourse import bacc
from concourse import bass_isa
from concourse.bass_utils import run_bass_kernel_spmd

F32 = mybir.dt.float32
F32R = mybir.dt.float32r
BF16 = mybir.dt.bfloat16
AF = mybir.ActivationFunctionType
AX = mybir.AxisListType
ALU = mybir.AluOpType

B, N_TOK, C = 8, 1024, 768
G = 8
H = 12
CT = C // 128               # 6 channel ptiles
NB = CT * G                 # 48 basis tiles
KT = CT + NB                # 54 contraction tiles (6 silu first, 48 basis)
GRID = np.linspace(-2.0, 2.0, G).astype(np.float64)
DENOM = 4.0 / 7.0
SQPI2 = math.sqrt(math.pi) / 2.0


def build_kernel(T=1024, sim_safe=False):
    TT = T // 128                       # token ptiles
    CHW = min(512, T)                   # matmul moving chunk width
    CH = T // CHW                       # chunks (2 at T=1024)
    VG = min(4, TT)                     # token tiles per v/proj pass
    v_passes = [list(range(i, min(i + VG, TT))) for i in range(0, TT, VG)]
    NP = 3                              # qk passes (4 ot's each)

    af_silu = AF.Sigmoid if sim_safe else AF.Silu
    af_derf = AF.Sigmoid if sim_safe else AF.Derivative_Erf
    af_exp = AF.Sigmoid if sim_safe else AF.Exp

    nc = bacc.Bacc("TRN2", target_bir_lowering=False, debug=False, num_devices=8)

    # ---- dram io ----
    xT_d = nc.dram_tensor("xT", (C, T), F32, kind="ExternalInput")
    w1qk_d = nc.dram_tensor("w1qk", (NB, 128, 1536), F32R, kind="ExternalInput")
    w1v_d = nc.dram_tensor("w1v", (NB, 128, 768), F32R, kind="ExternalInput")
    w2_d = nc.dram_tensor("w2", (NB, 128, 768), F32R, kind="ExternalInput")
    w1qks_d = nc.dram_tensor("w1qks", (CT, 128, 1536), BF16,
                             kind="ExternalInput")
    w1vs_d = nc.dram_tensor("w1vs", (CT, 128, 768), BF16,
                            kind="ExternalInput")
    w2s_d = nc.dram_tensor("w2s", (CT, 128, 768), BF16,
                           kind="ExternalInput")
    b1qk_d = nc.dram_tensor("b1qk", (12, 128), F32, kind="ExternalInput")
    b1v_d = nc.dram_tensor("b1v", (1, 768), F32, kind="ExternalInput")
    b2_d = nc.dram_tensor("b2", (1, 768), F32, kind="ExternalInput")
    asc1_d = nc.dram_tensor("asc1", (CT, 128), F32, kind="ExternalInput")
    abi1_d = nc.dram_tensor("abi1", (NB, 128), F32, kind="ExternalInput")
    asc2_d = nc.dram_tensor("asc2", (CT, 128), F32, kind="ExternalInput")
    abi2_d = nc.dram_tensor("abi2", (NB, 128), F32, kind="ExternalInput")
    out_d = nc.dram_tensor("out", (T, C), F32, kind="ExternalOutput")

    def _r(ap):
        return ap.bitcast(F32R)

    with tile.TileContext(nc) as tc:
        with tc.tile_pool(name="const", bufs=1) as const, \
             tc.tile_pool(name="potp", bufs=1) as potp, \
             tc.tile_pool(name="st2sb", bufs=1) as st2sb:

            # ---- constants ----
            asc1 = const.tile([128, CT], F32)
            abi1 = const.tile([128, NB], F32)
            asc2 = const.tile([128, CT], F32)
            abi2 = const.tile([128, NB], F32)
            nc.sync.dma_start(asc1[:], asc1_d.rearrange("c p -> p c"))
            nc.sync.dma_start(abi1[:], abi1_d.rearrange("k p -> p k"))
            nc.sync.dma_start(asc2[:], asc2_d.rearrange("c p -> p c"))
            nc.sync.dma_start(abi2[:], abi2_d.rearrange("k p -> p k"))
            b1qk = const.tile([128, 12], F32)
            nc.sync.dma_start(b1qk[:], b1qk_d.rearrange("o p -> p o"))
            b1v_b = const.tile([128, 768], F32)
            b2_b = const.tile([128, 768], F32)
            with tc.tile_pool(name="rowtmp", bufs=1) as rowtmp:
                b1v_row = rowtmp.tile([1, 768], F32)
                b2_row = rowtmp.tile([1, 768], F32)
                nc.sync.dma_start(b1v_row[:], b1v_d[:])
                nc.sync.dma_start(b2_row[:], b2_d[:])
                nc.gpsimd.partition_broadcast(b1v_b[:], b1v_row[:])
                nc.gpsimd.partition_broadcast(b2_b[:], b2_row[:])
            ones_bf = const.tile([128, 1], BF16)
            nc.vector.memset(ones_bf[:], 1.0)
            eps_t = const.tile([128, 1], F32)
            nc.vector.memset(eps_t[:], 1e-5)

            # ---- persistent activations ----
            OT = potp.tile([128, CT, T], BF16)        # attn out channel-major
            V = potp.tile([128, TT, 768], BF16)       # v token-major
            s2_row = st2sb.tile([1, T], F32)          # LN2 stat stash
            ss2_row = st2sb.tile([1, T], F32)

            def ln_wide_chain(s_row, ss_row, lnw):
                """[1,T] SBUF sums -> (rs_b, murs_b) [128,T] wide."""
                t1 = lnw.tile([128, T], F32, tag="t1")
                t2 = lnw.tile([128, T], F32, tag="t2")
                t3 = lnw.tile([128, T], F32, tag="t3")
                nc.gpsimd.partition_broadcast(t1[:], s_row[:])
                nc.gpsimd.partition_broadcast(t2[:], ss_row[:])
                nc.vector.tensor_scalar_mul(t1[:], t1[:], 1.0 / C)  # mean
                nc.vector.tensor_scalar_mul(t2[:], t2[:], 1.0 / C)  # E[x^2]
                nc.vector.tensor_mul(t3[:], t1[:], t1[:])
                nc.vector.tensor_sub(t2[:], t2[:], t3[:])           # var
                nc.scalar.activation(out=t3[:], in_=t2[:], func=AF.Sqrt,
                                     bias=eps_t[:, 0:1], scale=1.0)
                nc.vector.reciprocal(t2[:], t3[:])                  # rs
                nc.vector.tensor_mul(t1[:], t1[:], t2[:])           # mu*rs
                return t2, t1

            def emit_stats(mov_s, mov_ss, ps_s, ps_ss, first, last):
                for ch in range(CH):
                    sl = slice(ch * CHW, (ch + 1) * CHW)
                    nc.tensor.matmul(ps_s[:, sl], ones_bf[:], mov_s[:, sl],
                                     start=first, stop=last)
                    nc.tensor.matmul(ps_ss[:, sl], ones_bf[:], mov_ss[:, sl],
                                     start=first, stop=last)

            def basis_chunk(hT, asc, abi, kb, tok0, width, pool):
                """[128, width] f32r basis tile kb."""
                ct = kb % CT
                bt = pool.tile([128, width], F32R, tag="bt")
                nc.scalar.activation(out=bt[:],
                                     in_=hT[:, ct, tok0:tok0 + width],
                                     func=af_derf,
                                     scale=asc[:, ct:ct + 1],
                                     bias=abi[:, kb:kb + 1])
                return bt[:]

            def token_major_pass(hT, siluT, asc, abi, w_d, ws_d, sink, wpool,
                                 bpool, ps_pool, wtag):
                """basis-stationary matmul pass: sink(tt, psum_tile).

                k order: 6 silu tiles (bf16) first, then 48 basis (f32r)."""
                for tts in v_passes:
                    tok0 = tts[0] * 128
                    tokw = len(tts) * 128
                    psum = {tt: ps_pool.tile([128, 768], F32, tag="psv",
                                             name=f"ps_{wtag}_{tt}")
                            for tt in tts}
                    for k in range(KT):
                        if k < CT:
                            wt = wpool.tile([128, 768], BF16, tag=wtag + "s")
                            nc.sync.dma_start(wt[:], ws_d[k])
                            bt = siluT[:, k, tok0:tok0 + tokw]
                        else:
                            wt = wpool.tile([128, 768], F32R, tag=wtag)
                            nc.sync.dma_start(wt[:], w_d[k - CT])
                            bt = basis_chunk(hT, asc, abi, k - CT, tok0,
                                             tokw, bpool)
                        for i, tt in enumerate(tts):
                            lhs = bt[:, i * 128:(i + 1) * 128]
                            nc.tensor.matmul(psum[tt][:, 0:512], lhs,
                                             wt[:, 0:512],
                                             start=(k == 0), stop=(k == KT - 1))
                            nc.tensor.matmul(psum[tt][:, 512:768], lhs,
                                             wt[:, 512:768],
                                             start=(k == 0), stop=(k == KT - 1))
                    for tt in tts:
                        sink(tt, psum[tt])

            # ================= layer 1: LN + silu =================
            xpool = tc.tile_pool(name="xload", bufs=1)
            xp = xpool.__enter__()
            silu1_pool = tc.tile_pool(name="silu1", bufs=1)
            s1p = silu1_pool.__enter__()
            xT = xp.tile([128, CT, T], F32)           # becomes hT1 in place
            siluT1 = s1p.tile([128, CT, T], BF16)     # bf16 (small base path)
            with tc.tile_pool(name="ln1tmp", bufs=2) as ln1tmp, \
                 tc.tile_pool(name="ln1w", bufs=1) as ln1w, \
                 tc.tile_pool(name="ps_st1", bufs=1, space="PSUM") as ps_st1:
                xr6 = xT_d.rearrange("(ct p) t -> ct p t", p=128)
                for ct in range(CT):
                    nc.sync.dma_start(xT[:, ct], xr6[ct])
                for ct in range(CT):
                    nc.scalar.activation(out=siluT1[:, ct], in_=xT[:, ct],
                                         func=af_silu)
                ps_s = ps_st1.tile([1, T], F32, tag="ps_s")
                ps_ss = ps_st1.tile([1, T], F32, tag="ps_ss")
                for ct in range(CT):
                    xr = ln1tmp.tile([128, T], BF16, tag="xr")
                    nc.vector.tensor_copy(xr[:], xT[:, ct])
                    xsq = ln1tmp.tile([128, T], BF16, tag="xsq")
                    nc.vector.tensor_mul(xsq[:], xT[:, ct], xT[:, ct])
                    emit_stats(xr, xsq, ps_s, ps_ss, ct == 0, ct == CT - 1)
                s_row = ln1w.tile([1, T], F32, tag="s_row")
                ss_row = ln1w.tile([1, T], F32, tag="ss_row")
                nc.vector.tensor_copy(s_row[:], ps_s[:])
                nc.vector.tensor_copy(ss_row[:], ps_ss[:])
                rs_b, murs_b = ln_wide_chain(s_row, ss_row, ln1w)
                # hT1 in place of xT (silu + stats already consumed x)
                for ct in range(CT):
                    nc.vector.tensor_mul(xT[:, ct], xT[:, ct], rs_b[:])
                    nc.vector.tensor_sub(xT[:, ct], xT[:, ct], murs_b[:])
            hT1 = xT

            # ================= v pass =================
            with tc.tile_pool(name="w1vs", bufs=6) as w1vs, \
                 tc.tile_pool(name="bas_v", bufs=4) as bas_v, \
                 tc.tile_pool(name="ps_v", bufs=4, space="PSUM") as ps_v:
                def v_sink(tt, ps):
                    nc.vector.tensor_add(V[:, tt], ps[:], b1v_b[:])
                token_major_pass(hT1, siluT1, asc1, abi1, w1v_d, w1vs_d,
                                 v_sink, w1vs, bas_v, ps_v, "w1v")

            # ================= qk passes + attention =================
            with tc.tile_pool(name="qkTp", bufs=1) as qkTp, \
                 tc.tile_pool(name="ETp", bufs=1) as ETp, \
                 tc.tile_pool(name="denp", bufs=1) as denp, \
                 tc.tile_pool(name="w1s", bufs=6) as w1s, \
                 tc.tile_pool(name="bas_q", bufs=4) as bas_q, \
                 tc.tile_pool(name="ln2tmp", bufs=1) as ln2tmp:

                def emit_qk_mm(qkT, ps_at, i, c, mms):
                    """QK matmuls for pair-set i, chunk c -> (psum, exp) list.

                    Appends (psum_tile, et_ap) pairs to mms; caller issues
                    EXPs (ACT) right after each matmul so psum rotates."""
                    for u in (0, 1):          # hp = 2i+u; q slot 2u, k 2u+1
                        hp = 2 * i + u
                        for mt in range(TT):
                            for hh in (0, 1):
                                bp = hh * 64
                                pst = ps_at.tile([128, CHW], F32, tag="psat",
                                                 name=f"psat_{hp}_{mt}_{c}_{hh}")
                                nc.tensor.matmul(
                                    pst[:],
                                    qkT[bp:bp + 64, 2 * u + 1,
                                        mt * 128:(mt + 1) * 128],
                                    qkT[bp:bp + 64, 2 * u,
                                        c * CHW:(c + 1) * CHW],
                                    start=True, stop=True)
                                mms.append(((u, hh, c), mt, pst))

                def emit_exp(ET, key, mt, pst):
                    nc.scalar.activation(out=ET[key][:, mt, :], in_=pst[:],
                                         func=af_exp, scale=0.125)

                def emit_av(i, ET, ps_av, c):
                    """A@V for pair-set i chunk c + normalize into OT."""
                    for u in (0, 1):
                        hp = 2 * i + u
                        po = ps_av.tile([128, CHW], F32, tag="psav",
                                        name=f"psav_{hp}_{c}")
                        for hh in (0, 1):
                            h = 2 * hp + hh
                            bp = hh * 64
                            for kt in range(TT):
                                nc.tensor.matmul(
                                    po[bp:bp + 64],
                                    V[:, kt, h * 64:(h + 1) * 64],
                                    ET[(u, hh, c)][:, kt, :],
                                    start=(kt == 0), stop=(kt == TT - 1))
                        for hh in (0, 1):
                            bp = hh * 64
                            et = ET[(u, hh, c)]
                            den = denp.tile([128, CHW], F32, tag=f"den{hh}")
                            nc.vector.tensor_add(den[:], et[:, 0, :],
                                                 et[:, 1, :])
                            for kt in range(2, TT):
                                nc.vector.tensor_add(den[:], den[:],
                                                     et[:, kt, :])
                            allr = denp.tile([128, CHW], F32, tag=f"allr{hh}")
                            nc.gpsimd.partition_all_reduce(
                                allr[:], den[:], channels=128,
                                reduce_op=bass_isa.ReduceOp.add)
                            nc.vector.reciprocal(allr[bp:bp + 64],
                                                 allr[bp:bp + 64])
                            nc.vector.tensor_mul(
                                OT[bp:bp + 64, hp, c * CHW:(c + 1) * CHW],
                                po[bp:bp + 64], allr[bp:bp + 64])

                ET = {(u, hh, c): ETp.tile([128, TT, CHW], BF16,
                                           tag=f"ET{u}{hh}{c}",
                                           name=f"ET_{u}_{hh}_{c}")
                      for u in (0, 1) for hh in (0, 1) for c in range(CH)}

                def pass_qk(i):
                    qkT = qkTp.tile([128, 4, T], F32R, tag="qkT",
                                    name=f"qkT_{i}")
                    with tc.tile_pool(name=f"ps_qk{i}", bufs=8,
                                      space="PSUM") as ps_qk:
                        ps = {(j, c): ps_qk.tile([128, CHW], F32, tag="psqk",
                                                 name=f"psqk_{i}_{j}_{c}")
                              for j in range(4) for c in range(CH)}
                        for k in range(KT):
                            if k < CT:
                                wt = w1s.tile([128, 512], BF16, tag="w1ts")
                                nc.sync.dma_start(
                                    wt[:],
                                    w1qks_d[k, :, i * 512:(i + 1) * 512])
                            else:
                                wt = w1s.tile([128, 512], F32R, tag="w1t")
                                nc.sync.dma_start(
                                    wt[:],
                                    w1qk_d[k - CT, :, i * 512:(i + 1) * 512])
                            for c in range(CH):
                                if k < CT:
                                    bt = siluT1[:, k, c * CHW:(c + 1) * CHW]
                                else:
                                    bt = basis_chunk(hT1, asc1, abi1, k - CT,
                                                     c * CHW, CHW, bas_q)
                                for j in range(4):
                                    nc.tensor.matmul(
                                        ps[(j, c)][:],
                                        wt[:, j * 128:(j + 1) * 128],
                                        bt,
                                        start=(k == 0), stop=(k == KT - 1))
                        for j in range(4):
                            for c in range(CH):
                                nc.vector.tensor_scalar_add(
                                    qkT[:, j, c * CHW:(c + 1) * CHW],
                                    ps[(j, c)][:],
                                    b1qk[:, 4 * i + j:4 * i + j + 1])
                    return qkT

                def window(i, qkT, prev, ps_at, ps_av):
                    """AV(prev) interleaved with QK(i) matmuls + EXPs."""
                    mms = []
                    if prev is not None:
                        emit_av(prev, ET, ps_av, 0)
                    emit_qk_mm(qkT, ps_at, i, 0, mms)
                    for c in range(1, CH):
                        if prev is not None:
                            emit_av(prev, ET, ps_av, c)
                        emit_qk_mm(qkT, ps_at, i, c, mms)
                    for key, mt, pst in mms:
                        emit_exp(ET, key, mt, pst)

                qkT = pass_qk(0)
                with tc.tile_pool(name="ps_at0", bufs=4, space="PSUM") as at0, \
                     tc.tile_pool(name="ps_av0", bufs=2, space="PSUM") as av0:
                    window(0, qkT, None, at0, av0)
                qkT = pass_qk(1)
                with tc.tile_pool(name="ps_at1", bufs=4, space="PSUM") as at1, \
                     tc.tile_pool(name="ps_av1", bufs=2, space="PSUM") as av1:
                    window(1, qkT, 0, at1, av1)
                qkT = pass_qk(2)
                # last window + tail share pools so the LN2 stat matmuls can
                # fill the bubble while EXP(2) finishes on ACT
                with tc.tile_pool(name="ps_at2", bufs=2, space="PSUM") as at2, \
                     tc.tile_pool(name="ps_av2", bufs=2, space="PSUM") as av2, \
                     tc.tile_pool(name="ps_st2", bufs=1,
                                  space="PSUM") as ps_st2:
                    window(2, qkT, 1, at2, av2)
                    ps2_s = ps_st2.tile([1, T], F32, tag="ps2_s")
                    ps2_ss = ps_st2.tile([1, T], F32, tag="ps2_ss")

                    def emit_stats2(hp):
                        sq = ln2tmp.tile([128, T], BF16, tag="sq")
                        nc.vector.tensor_mul(sq[:], OT[:, hp], OT[:, hp])
                        emit_stats(OT[:, hp], sq, ps2_s, ps2_ss,
                                   hp == 0, hp == 5)

                    for hp in range(4):
                        emit_stats2(hp)
                    for c in range(CH):
                        emit_av(2, ET, av2, c)
                    for hp in (4, 5):
                        emit_stats2(hp)
                    nc.vector.tensor_copy(s2_row[:], ps2_s[:])
                    nc.vector.tensor_copy(ss2_row[:], ps2_ss[:])

            silu1_pool.__exit__(None, None, None)
            xpool.__exit__(None, None, None)

            # ================= layer 2 (proj) =================
            with tc.tile_pool(name="silu2", bufs=1) as s2p, \
                 tc.tile_pool(name="hT2p", bufs=1) as hT2p, \
                 tc.tile_pool(name="ln2w", bufs=1) as ln2w:
                siluT2 = s2p.tile([128, CT, T], BF16)
                for ct in range(CT):
                    nc.scalar.activation(out=siluT2[:, ct], in_=OT[:, ct],
                                         func=af_silu)
                rs2_b, murs2_b = ln_wide_chain(s2_row, ss2_row, ln2w)
                hT2 = hT2p.tile([128, CT, T], F32)
                for ct in range(CT):
                    nc.vector.tensor_mul(hT2[:, ct], OT[:, ct], rs2_b[:])
                    nc.vector.tensor_sub(hT2[:, ct], hT2[:, ct], murs2_b[:])

                with tc.tile_pool(name="w2s", bufs=6) as w2s, \
                     tc.tile_pool(name="bas_p", bufs=4) as bas_p, \
                     tc.tile_pool(name="outst", bufs=3) as outst, \
                     tc.tile_pool(name="ps_p", bufs=4, space="PSUM") as ps_p:
                    out_r = out_d.rearrange("(tt p) o -> tt p o", p=128)

                    def p_sink(tt, ps):
                        ob = outst.tile([128, 768], F32, tag="ob")
                        nc.vector.tensor_add(ob[:], ps[:], b2_b[:])
                        nc.sync.dma_start(out_r[tt], ob[:])
                    token_major_pass(hT2, siluT2, asc2, abi2, w2_d, w2s_d,
                                     p_sink, w2s, bas_p, ps_p, "w2")

    nc.compile()
    return nc


def host_prep(inputs, T=1024):
    """Build per-core input maps from the full (unsharded) inputs."""
    x = np.asarray(inputs["x"], dtype=np.float32)

    import ml_dtypes

    def pack_layer(spline_w, base_w, ln_w, ln_b):
        spline_w = np.asarray(spline_w, dtype=np.float64)
        base_w = np.asarray(base_w, dtype=np.float64)
        O = spline_w.shape[1]
        WS = np.empty((CT, 128, O), dtype=np.float64)   # silu/base tiles
        W = np.empty((NB, 128, O), dtype=np.float64)    # basis tiles
        for ct in range(CT):
            WS[ct] = base_w[ct * 128:(ct + 1) * 128]
        for g in range(G):
            sg = spline_w[g::G] * SQPI2          # [768, O]
            for ct in range(CT):
                W[g * CT + ct] = sg[ct * 128:(ct + 1) * 128]
        ln_w = np.asarray(ln_w, dtype=np.float64)
        ln_b = np.asarray(ln_b, dtype=np.float64)
        asc = (ln_w / DENOM).reshape(CT, 128).astype(np.float32)
        abi = np.empty((NB, 128), dtype=np.float32)
        for g in range(G):
            for ct in range(CT):
                abi[g * CT + ct] = \
                    ((ln_b - GRID[g]) / DENOM)[ct * 128:(ct + 1) * 128]
        return W.astype(np.float32), WS, asc, abi

    W1, WS1, asc1, abi1 = pack_layer(inputs["qkv_spline_w"],
                                     inputs["qkv_base_w"],
                                     inputs["qkv_ln_w"], inputs["qkv_ln_b"])
    W2, WS2, asc2, abi2 = pack_layer(inputs["proj_spline_w"],
                                     inputs["proj_base_w"],
                                     inputs["proj_ln_w"], inputs["proj_ln_b"])
    b1 = np.asarray(inputs["qkv_base_b"], dtype=np.float32)
    b2 = np.asarray(inputs["proj_base_b"], dtype=np.float32)

    # paired qk weight layout: [q(hp), k(hp)] contiguous per head pair
    def pair_qk(Wfull, dtype):
        w = np.empty((Wfull.shape[0], 128, 1536), dtype=dtype)
        for hp in range(6):
            w[:, :, hp * 256:hp * 256 + 128] = \
                Wfull[:, :, hp * 128:(hp + 1) * 128]
            w[:, :, hp * 256 + 128:(hp + 1) * 256] = \
                Wfull[:, :, 768 + hp * 128:768 + (hp + 1) * 128]
        return w

    b1qk = np.empty((12, 128), dtype=np.float32)
    for hp in range(6):
        b1qk[2 * hp] = b1[hp * 128:(hp + 1) * 128]
        b1qk[2 * hp + 1] = b1[768 + hp * 128:768 + (hp + 1) * 128]

    shared = {
        "w1qk": pair_qk(W1[:, :, :1536], np.float32),
        "w1qks": pair_qk(WS1[:, :, :1536], ml_dtypes.bfloat16),
        "w1v": np.ascontiguousarray(W1[:, :, 1536:]),
        "w1vs": WS1[:, :, 1536:].astype(ml_dtypes.bfloat16),
        "w2": np.ascontiguousarray(W2),
        "w2s": WS2.astype(ml_dtypes.bfloat16),
        "b1qk": b1qk,
        "b1v": b1[1536:].reshape(1, 768).copy(),
        "b2": b2.reshape(1, 768).copy(),
        "asc1": asc1, "abi1": abi1, "asc2": asc2, "abi2": abi2,
    }
    in_maps = []
    for core in range(x.shape[0]):
        m = dict(shared)
        m["xT"] = np.ascontiguousarray(x[core, :T].T)
        in_maps.append(m)
    return in_maps


_NC_CACHE = {}


def _get_nc(T=1024, sim_safe=False):
    key = (T, sim_safe)
    if key not in _NC_CACHE:
        _NC_CACHE[key] = build_kernel(T, sim_safe=sim_safe)
    return _NC_CACHE[key]


def kernel(**inputs) -> np.ndarray:
    nc = _get_nc()
    in_maps = host_prep(inputs)
    res = run_bass_kernel_spmd(nc, in_maps, core_ids=list(range(8)))
    out = np.stack([res.results[c]["out"] for c in range(len(in_maps))])
    return out.astype(np.float32)


if __name__ == "__main__":
    data = np.load("/root/problem/ref_data.npz")
    inputs = {k[3:]: data[k] for k in data.files if k.startswith("in_")}
    expected = data["expected64"]
    actual = kernel(**inputs)
    err = np.abs(actual - expected)
    print("absmax err:", err.max(),
          "rel2max:", err.max() / np.abs(expected).max())
    print("rel l2:",
          np.linalg.norm(actual - expected) / np.linalg.norm(expected))
